# revision 33
# baseline (speedup 1.0000x reference)
"""AttnBlock (GroupNorm -> QKV -> full attention -> proj + residual) on 8
Trainium2 NeuronCores, data-parallel over the batch dimension (b=8, one
sample per core).

fp8 (TRN e4m3, max-normal 240) DoubleRow design, transpose-free:
  h8  = fp8(16*GN(x)) from a bf16 copy of x (stats + apply); the fp32 x
        is DMAed in parallel (gpsimd queue) and used only for the residual.
  u8  = fp8(256*(A.T h + g)), A = (wq.T wk)/sqrt(c), g = wk.T bq /sqrt(c)
  sT  = h8.T u8 (scoresT layout: j on partitions, i free)  [DoubleRow]
  e8  = fp8(exp(s - 1.5))  (offset cancels in softmax; keeps e8 < 240)
  vp8 = fp8(16*(wp wv h).T)  [j-part, c free]
  S   = ones16.T e8 (PSUM = 16*rowsum, replicated on all partitions)
  O   = vp8.T e8 (PSUM = 16*unnormalized attn out)
  out = O * reciprocal(S) + (x + bp_eff)   (scales cancel exactly)
GN stats are split DVE (bn_stats, tiles 0/1) / ACT (Square+Ident accum,
tiles 2/3) and the group reduction runs vectorized over all 4 tiles.
"""

import functools

import numpy as np

B = 8
C = 512
W = 2048
G = 32
EPS = 1e-6
P = 128
CT = C // P          # 4 channel tiles
NW = W // 512        # 4 w-chunks of 512
IT = W // P          # 16 j-tiles

AH = 16.0            # h8 = AH * h
AA = 8192.0          # A8 = AA * A
AWV = 256.0          # WPV8T = AWV * (wp wv).T
AU = 256.0           # u8 = AU * (u + g)
AV = 16.0            # vp8 = AV * vp ; S lhsT ones = AV too (cancel)
EXP_OFF = 1.5
SC_EXP = 1.0 / (AH * AU)
SC_U = AU / (AA * AH)
SC_V = AV / (AWV * AH)

TRACE = False
LAST_EXEC_NS = None
LAST_TRACE_PATH = None


def _build_nc(with_bias=False):
    import concourse.bass as bass
    import concourse.mybir as mybir
    import concourse.tile as tile
    from concourse import bacc

    f32 = mybir.dt.float32
    bf16 = mybir.dt.bfloat16
    f8 = mybir.dt.float8e4
    u8dt = mybir.dt.uint8
    Ident = mybir.ActivationFunctionType.Identity
    Exp = mybir.ActivationFunctionType.Exp
    Sqrt = mybir.ActivationFunctionType.Sqrt
    Square = mybir.ActivationFunctionType.Square
    mult = mybir.AluOpType.mult
    add = mybir.AluOpType.add
    subtract = mybir.AluOpType.subtract
    DR = mybir.MatmulPerfMode.DoubleRow

    nc = bacc.Bacc()

    x_d = nc.declare_dram_parameter("x", [C, W], f32, isOutput=False)
    x16_d = nc.declare_dram_parameter("x16", [C, W], bf16, isOutput=False)
    w8_d = nc.declare_dram_parameter("w8", [P, 4352], u8dt, isOutput=False)
    aux_d = nc.declare_dram_parameter("aux", [P, 1040], f32, isOutput=False)
    out_d = nc.declare_dram_parameter("out", [C, W], f32, isOutput=True)

    with tile.TileContext(nc) as tc:
        with (
            tc.tile_pool(name="big", bufs=1) as big,
            tc.tile_pool(name="gn", bufs=2) as gnp,
            tc.tile_pool(name="ot", bufs=2) as otp,
        ):
            w8_sb = big.tile([P, 4352], f8, name="w8")
            aux_sb = big.tile([P, 1040], f32, name="aux")
            x16_sb = big.tile([P, CT, W], bf16, name="x16")
            x_sb = [big.tile([P, W], f32, name=f"x{t}") for t in range(CT)]
            h8 = big.tile([P, CT, W], f8, name="h8")
            u8 = big.tile([P, CT, W], f8, name="u8")
            e8 = big.tile([P, IT, W], f8, name="e8")
            vp8 = big.tile([P, IT, C], f8, name="vp8")
            rec_sb = big.tile([P, W], f32, name="rec")
            scr16 = big.tile([P, W], bf16, name="scr16")
            eps_t = big.tile([P, 1], f32, name="eps")
            nc.vector.memset(eps_t, EPS)
            off_t = big.tile([P, 1], f32, name="off")
            nc.vector.memset(off_t, -EXP_OFF)

            # DMA order: x16 first (gates GN stats), then aux/w8, then the
            # residual x (only needed at the epilogue).
            for t in range(CT):
                nc.sync.dma_start(out=x16_sb[:, t, :],
                                  in_=x16_d[t * P:(t + 1) * P, :])
            nc.sync.dma_start(out=aux_sb, in_=aux_d[:, :])
            nc.sync.dma_start(out=w8_sb, in_=w8_d[:, :].bitcast(f8))
            for t in range(CT):
                nc.sync.dma_start(out=x_sb[t], in_=x_d[t * P:(t + 1) * P, :])

            a8 = w8_sb[:, 0:2048].rearrange("p (t o) -> p t o", t=CT)
            wpv8 = w8_sb[:, 2048:4096].rearrange("p (t o) -> p t o", t=CT)
            ones8 = w8_sb[:, 4096:4352].rearrange("p (s m) -> p s m", s=2)
            s_sel = aux_sb[:, 0:512].rearrange("p (t g) -> p t g", t=CT)
            st_sel = aux_sb[:, 512:1024].rearrange("p (t c) -> p t c", t=CT)
            gam16 = aux_sb[:, 1024:1028]
            bet16 = aux_sb[:, 1028:1032]
            g256 = aux_sb[:, 1032:1036]
            bp_ap = aux_sb[:, 1036:1040]

            # ===== GroupNorm stats: tiles 0/1/3 on DVE, 2 on ACT =====
            st2_l = [gnp.tile([P, 2], f32, tag=f"st2_{t}", name=f"st2_{t}")
                     for t in range(CT)]
            for t in (2,):
                sums = gnp.tile([P, 2], f32, tag=f"sums{t}", name=f"sums{t}")
                nc.scalar.activation(out=scr16, in_=x16_sb[:, t, :],
                                     func=Square, accum_out=sums[:, 1:2])
                nc.scalar.activation(out=scr16, in_=x16_sb[:, t, :],
                                     func=Ident, accum_out=sums[:, 0:1])
                nc.vector.tensor_scalar_mul(st2_l[t], sums, 1.0 / W)
            for t in (0, 1, 3):
                st2 = st2_l[t]
                stats = gnp.tile([P, NW, 6], f32, tag="bnstats", name=f"bns{t}")
                for sg in range(NW):
                    nc.vector.bn_stats(out=stats[:, sg, :],
                                       in_=x16_sb[:, t, sg * 512:(sg + 1) * 512])
                mv = gnp.tile([P, 2], f32, tag="mv", name=f"mv{t}")
                nc.vector.bn_aggr(out=mv, in_=stats)
                nc.vector.tensor_copy(out=st2[:, 0:1], in_=mv[:, 0:1])
                nc.vector.tensor_tensor(out=st2[:, 1:2], in0=mv[:, 0:1],
                                        in1=mv[:, 0:1], op=mult)
                nc.vector.tensor_add(out=st2[:, 1:2], in0=st2[:, 1:2],
                                     in1=mv[:, 1:2])

            # ===== group reduce + affine, vectorized over tiles =====
            gn_ps_cm = tc.tile_pool(name="gn_ps", bufs=2, space="PSUM")
            gn_ps = gn_ps_cm.__enter__()
            ps_g = gn_ps.tile([P, 8], f32, tag="gnps", name="ps_g")
            for t in range(CT):
                nc.tensor.matmul(ps_g[:, 2 * t:2 * t + 2], lhsT=s_sel[:, t, :],
                                 rhs=st2_l[t], start=True, stop=True)
            gsr = gnp.tile([P, 8], f32, tag="gsr", name="gsr")
            nc.vector.tensor_copy(out=gsr[:8, :], in_=ps_g[:8, :])
            grv = gsr.rearrange("p (t two) -> p t two", t=CT)
            gs2 = gnp.tile([P, 8], f32, tag="gs2", name="gs2")
            gsv = gs2.rearrange("p (t two) -> p t two", t=CT)
            nc.vector.memset(gs2, 0.0)
            nc.vector.tensor_copy(out=gsv[:8, :, 0], in_=grv[:8, :, 0])
            nc.vector.tensor_tensor(out=gsv[:8, :, 1], in0=grv[:8, :, 0],
                                    in1=grv[:8, :, 0], op=mult)
            nc.vector.tensor_tensor(out=gsv[:8, :, 1], in0=grv[:8, :, 1],
                                    in1=gsv[:8, :, 1], op=subtract)
            nc.scalar.activation(out=gsv[:8, :, 1], in_=gsv[:8, :, 1],
                                 func=Sqrt, bias=eps_t[:8], scale=1.0)
            nc.vector.reciprocal(gsv[:8, :, 1], gsv[:8, :, 1])
            ps_bc = gn_ps.tile([P, 8], f32, tag="gnps", name="ps_bc")
            for t in range(CT):
                nc.tensor.matmul(ps_bc[:, 2 * t:2 * t + 2],
                                 lhsT=st_sel[:, t, :],
                                 rhs=gs2[:, 2 * t:2 * t + 2],
                                 start=True, stop=True)
            bcv = ps_bc.rearrange("p (t two) -> p t two", t=CT)
            alph = gnp.tile([P, CT], f32, tag="alph", name="alph")
            beta = gnp.tile([P, CT], f32, tag="beta", name="beta")
            nc.vector.tensor_tensor(out=alph, in0=bcv[:, :, 1], in1=gam16,
                                    op=mult)
            nc.vector.tensor_tensor(out=beta, in0=bcv[:, :, 0], in1=alph,
                                    op=mult)
            nc.vector.tensor_tensor(out=beta, in0=bet16, in1=beta, op=subtract)
            for t in range(CT):
                if t % 2 == 0:
                    nc.scalar.activation(out=h8[:, t, :], in_=x16_sb[:, t, :],
                                         func=Ident, scale=alph[:, t:t + 1],
                                         bias=beta[:, t:t + 1])
                else:
                    nc.vector.tensor_scalar(out=h8[:, t, :],
                                            in0=x16_sb[:, t, :],
                                            scalar1=alph[:, t:t + 1],
                                            scalar2=beta[:, t:t + 1],
                                            op0=mult, op1=add)
            gn_ps_cm.__exit__(None, None, None)

            # ===== u = A.T h (+g) [ACT cast], vp = (wp wv h).T [DVE cast].
            # u slabs first (casts overlap the vp chains); vp casts split in
            # two so the scores pool's bank reuse never waits long. =====
            mm_ps_cm = tc.tile_pool(name="mm_ps", bufs=2, space="PSUM")
            mm_ps = mm_ps_cm.__enter__()

            def u_mm(slab, k, ic, pr):
                nc.tensor.matmul(
                    slab[:, ic * 512:(ic + 1) * 512],
                    lhsT=a8[:, 2 * pr:2 * pr + 2, k * P:(k + 1) * P],
                    rhs=h8[:, 2 * pr:2 * pr + 2, ic * 512:(ic + 1) * 512],
                    start=(pr == 0), stop=(pr == 1), perf_mode=DR)

            # u slabs 0/1: all pr0 matmuls first (they need only h8 tiles
            # 0/1, which the applies finish ~2us before tiles 2/3)
            u01 = [mm_ps.tile([P, W], f32, tag="slab", name=f"u_ps{k}")
                   for k in range(2)]
            for k in range(2):
                for ic in range(NW):
                    u_mm(u01[k], k, ic, 0)
            for k in range(2):
                for ic in range(NW):
                    u_mm(u01[k], k, ic, 1)
                nc.scalar.activation(out=u8[:, k, :], in_=u01[k], func=Ident,
                                     scale=SC_U, bias=g256[:, k:k + 1])
            for k in range(2, CT):
                slab = mm_ps.tile([P, W], f32, tag="slab", name=f"u_ps{k}")
                for ic in range(NW):
                    for pr in range(2):
                        u_mm(slab, k, ic, pr)
                nc.scalar.activation(out=u8[:, k, :], in_=slab, func=Ident,
                                     scale=SC_U, bias=g256[:, k:k + 1])
            for k in range(CT):
                slab = mm_ps.tile([P, W], f32, tag="slab", name=f"vp_ps{k}")
                for j4 in range(4):
                    jt = k * 4 + j4
                    for pr in range(2):
                        nc.tensor.matmul(
                            slab[:, j4 * 512:(j4 + 1) * 512],
                            lhsT=h8[:, 2 * pr:2 * pr + 2, jt * P:(jt + 1) * P],
                            rhs=wpv8[:, 2 * pr:2 * pr + 2, :],
                            start=(pr == 0), stop=(pr == 1), perf_mode=DR)
                # casts: first half on DVE (overlaps later chains), last two
                # slabs on the otherwise-idle ACT so the trailing cast is
                # short and the scores pool starts promptly
                if k < 2:
                    nc.vector.tensor_scalar_mul(
                        vp8[:, k * 4:(k + 1) * 4, :],
                        slab.rearrange("p (a b) -> p a b", a=4), SC_V)
                else:
                    nc.scalar.activation(
                        out=vp8[:, k * 4:(k + 1) * 4, :],
                        in_=slab.rearrange("p (a b) -> p a b", a=4),
                        func=Ident, scale=SC_V)
            mm_ps_cm.__exit__(None, None, None)

            # ===== scoresT + exp: sT[j,i] = sum_c h8[c,j] u8[c,i] =====
            sc_ps_cm = tc.tile_pool(name="sc_ps", bufs=2, space="PSUM")
            sc_ps = sc_ps_cm.__enter__()
            for jt in range(IT):
                slab = sc_ps.tile([P, W], f32, tag="sc", name=f"sc{jt}")
                for ic in range(NW):
                    for pr in range(2):
                        nc.tensor.matmul(
                            slab[:, ic * 512:(ic + 1) * 512],
                            lhsT=h8[:, 2 * pr:2 * pr + 2, jt * P:(jt + 1) * P],
                            rhs=u8[:, 2 * pr:2 * pr + 2, ic * 512:(ic + 1) * 512],
                            start=(pr == 0), stop=(pr == 1), perf_mode=DR)
                nc.scalar.activation(out=e8[:, jt, :], in_=slab, func=Exp,
                                     scale=SC_EXP, bias=off_t)
            sc_ps_cm.__exit__(None, None, None)

            # ===== S (replicated row sums), out chains, chunked epilogue ====
            o_ps_cm = tc.tile_pool(name="o_ps", bufs=2, space="PSUM")
            o_ps = o_ps_cm.__enter__()
            sl_s = o_ps.tile([P, W], f32, tag="ops", name="s_ps")
            for ic in range(NW):
                for jp in range(8):
                    nc.tensor.matmul(
                        sl_s[:, ic * 512:(ic + 1) * 512],
                        lhsT=ones8,
                        rhs=e8[:, 2 * jp:2 * jp + 2, ic * 512:(ic + 1) * 512],
                        start=(jp == 0), stop=(jp == 7), perf_mode=DR)
                nc.vector.reciprocal_approx_fast(
                    out=rec_sb[:, ic * 512:(ic + 1) * 512],
                    in_=sl_s[:, ic * 512:(ic + 1) * 512])
            for ct in range(CT):
                sl_o = o_ps.tile([P, W], f32, tag="ops", name=f"o_ps{ct}")
                t_sb = otp.tile([P, W], f32, tag="t", name=f"t{ct}")
                osb = otp.tile([P, W], f32, tag="osb", name=f"osb{ct}")
                for ic in range(NW):
                    sl = slice(ic * 512, (ic + 1) * 512)
                    for jp in range(8):
                        nc.tensor.matmul(
                            sl_o[:, sl],
                            lhsT=vp8[:, 2 * jp:2 * jp + 2, ct * P:(ct + 1) * P],
                            rhs=e8[:, 2 * jp:2 * jp + 2, sl],
                            start=(jp == 0), stop=(jp == 7), perf_mode=DR)
                    nc.vector.tensor_tensor(out=t_sb[:, sl], in0=sl_o[:, sl],
                                            in1=rec_sb[:, sl], op=mult)
                    if with_bias:
                        nc.vector.tensor_add(out=t_sb[:, sl], in0=t_sb[:, sl],
                                             in1=x_sb[ct][:, sl])
                        nc.scalar.activation(out=osb[:, sl], in_=t_sb[:, sl],
                                             func=Ident, scale=1.0,
                                             bias=bp_ap[:, ct:ct + 1])
                    else:
                        nc.vector.tensor_add(out=osb[:, sl], in0=t_sb[:, sl],
                                             in1=x_sb[ct][:, sl])
                    nc.sync.dma_start(out=out_d[ct * P:(ct + 1) * P, sl],
                                      in_=osb[:, sl])
            o_ps_cm.__exit__(None, None, None)

    nc.finalize()
    return nc


@functools.lru_cache(maxsize=2)
def _built(with_bias=False):
    return _build_nc(with_bias)


def _fp8(v, scale):
    import ml_dtypes
    a = np.asarray(v, np.float32) * np.float32(scale)
    m = float(np.abs(a).max()) if a.size else 0.0
    assert m <= 239.0, f"fp8 overflow: absmax {m}"
    return np.ascontiguousarray(a.astype(ml_dtypes.float8_e4m3fn))


def kernel(x, gn_gamma, gn_beta, wq, bq, wk, bk, wv, bv, wp, bp):
    global LAST_EXEC_NS, LAST_TRACE_PATH
    import ml_dtypes
    from concourse.bass_utils import run_bass_kernel_spmd

    f = np.float32
    f64 = np.float64
    x = np.asarray(x, f)
    wq64 = np.asarray(wq, f64)
    wk64 = np.asarray(wk, f64)
    wv64 = np.asarray(wv, f64)
    wp64 = np.asarray(wp, f64)
    scale = float(C) ** -0.5

    A = (wq64.T @ wk64) * scale                       # (c_in, c_out)
    WPVT = (wp64 @ wv64).T                            # (c_in, c_out)
    g = (wk64.T @ (np.asarray(bq, f64) * scale))      # (c,)
    bp_eff = (np.asarray(bp, f64) + wp64 @ np.asarray(bv, f64)).astype(f)

    def pmaj3(m, sc):
        # (C, C) -> [P, CT, C] fp8 with row p holding c_in = t*P + p
        return _fp8(np.asarray(m, f).reshape(CT, P, C).transpose(1, 0, 2), sc)

    w8 = np.zeros((P, 4352), dtype=np.uint8)
    w8[:, 0:2048] = pmaj3(A, AA).reshape(P, 2048).view(np.uint8)
    w8[:, 2048:4096] = pmaj3(WPVT, AWV).reshape(P, 2048).view(np.uint8)
    w8[:, 4096:4352] = np.full((P, 256), AV,
                               dtype=ml_dtypes.float8_e4m3fn).view(np.uint8)

    gsz = C // G
    aux = np.zeros((P, 1040), dtype=f)
    pidx = np.arange(P)
    for t in range(CT):
        aux[pidx, t * P + pidx // gsz] = 1.0 / gsz          # S selector
        aux[pidx // gsz, 512 + t * P + pidx] = 1.0          # ST selector
    aux[:, 1024:1028] = (AH * np.asarray(gn_gamma, f)).reshape(CT, P).T
    aux[:, 1028:1032] = (AH * np.asarray(gn_beta, f)).reshape(CT, P).T
    aux[:, 1032:1036] = (AU * g).astype(f).reshape(CT, P).T
    aux[:, 1036:1040] = bp_eff.reshape(CT, P).T

    shared = dict(w8=w8, aux=aux)
    in_maps = []
    for i in range(B):
        xi = np.ascontiguousarray(x[i])
        in_maps.append(dict(x=xi, x16=xi.astype(ml_dtypes.bfloat16), **shared))

    nc = _built(bool(np.any(bp_eff != 0)))
    for attempt in range(3):
        try:
            res = run_bass_kernel_spmd(nc, in_maps, list(range(B)), trace=TRACE)
            out = np.stack([np.asarray(res.results[i]["out"], dtype=f)
                            for i in range(B)], axis=0)
            break
        except Exception:  # transient NRT device errors: retry
            if attempt == 2:
                raise
            import time
            time.sleep(2.0)
    if TRACE:
        LAST_EXEC_NS = res.exec_time_ns
        if res.instructions_and_trace is not None:
            LAST_TRACE_PATH = res.instructions_and_trace[1]
    return out
